# revision 60
# baseline (speedup 1.0000x reference)
"""Trainium2 Bass kernel for nn_DGN4 (gnn_message_passing)  -- v3.

Reference semantics (B=4, T=2048, D=256, K_SIM=8, K_CON=4):
  xn    = x / max(||x||, 1e-12)                       (row L2-normalize)
  sim   = xn @ xn^T, causally masked (strictly past), masked = -1e9
  A_sim = top-8 per row (one-hot), zeroed outside past
  A_con = "bottom-4" of sim excluding A_sim -- future columns score +1e9
          in the negated space, so only rows T-3..T-1 get 1..3 real
          con-neighbors; everywhere else A_con == 0.
  msg_* = degree-normalized mean of selected x rows
  ctx   = alpha*msg_pos + (1-alpha)*msg_neg
  delta = gelu(mix*x + (1-mix)*ctx) * scale   (exact erf gelu)

Sharding: 8 cores = 4 batches x 2 row-shards, one uniform SPMD program;
per-core differences are data only (odd cores get adjacent 128-row blocks
swapped so the same static tile offsets address their rows).

v3 structure (vs v2):
  - xn and its transpose are precomputed on the host and shipped as xnT
    (f16): the on-device norm/normalize/PE-transpose/psum-copy prologue
    is gone.
  - A^T is produced by ONE batched DMA-engine xbar transpose per tile
    (3D contiguous out => per-128-block transposes); PE transposes and
    the psum->sbuf copies of A^T are gone.  NOTE: the xbar destination
    MUST be a contiguous tile -- a padded/sliced dst is silently wrong
    on hardware and breaks dependency tracking.
  - top-8 runs chunk-wise on DVE directly from PSUM, in parallel with
    the Act psum->w copies, so selection overlaps the sim matmuls.
  - outputs are bf16, written in pipeline order and DMA'd in pairs.
  - selection thresholds stay fp32 (f16 sim values would tie at the
    8th-largest and break the degree normalization).
"""

import numpy as np

B, T, D = 4, 2048, 256
PB = 128                 # partition block
NBLK = T // PB           # 16 row/col blocks per batch
NTILE = 8                # program tiles per core
# width (in 128-blocks) and own-block index per program tile; widths pair to 18
WB = [2, 16, 4, 14, 6, 12, 8, 10]
OWN = [0, 15, 2, 13, 4, 11, 6, 9]
NEGF = -60000.0

# tile processing order (pipeline): tile 0/2 need only the first x blocks
# (earliest PE start), tile 1 (full width + contrarian chain) early so its
# long select pipeline overlaps the rest.
TORD = [0, 2, 1, 3, 5, 7, 6, 4]
# agg emission order: tile 1 (whose contrarian gather the PE must wait on)
# goes last so the in-order PE queue never stalls mid-pipeline
AGG_ORD = [0, 2, 3, 5, 7, 6, 4, 1]

_PROGRAMS = {}


def _build_patterns():
    """Penalty patterns (0 = keep, NEGF = masked) for the last two 128-col
    blocks of each program tile, as a function of tile parity (k%2) and
    core parity.  patterns[parity] is [2, PB, 2*PB] float32."""
    tri = np.where(np.arange(PB)[None, :] < np.arange(PB)[:, None], 0.0, NEGF)
    keep = np.zeros((PB, PB), np.float32)
    mask = np.full((PB, PB), NEGF, np.float32)
    out = []
    for parity in (0, 1):
        m = np.zeros((2, PB, 2 * PB), np.float32)
        if parity == 0:
            m[0] = np.concatenate([tri, mask], axis=1)   # even k
            m[1] = np.concatenate([keep, tri], axis=1)   # odd k
        else:
            m[0] = np.concatenate([tri, keep], axis=1)
            m[1] = np.concatenate([mask, tri], axis=1)
        out.append(m.astype(np.float32))
    return out


def _build_program(unit_affine=True):
    import concourse.bacc as bacc
    import concourse.tile as tile
    from concourse import mybir
    from concourse.tile_rust import add_dep_helper

    f32 = mybir.dt.float32
    f16 = mybir.dt.float16
    bf16 = mybir.dt.bfloat16
    Alu = mybir.AluOpType
    Act = mybir.ActivationFunctionType

    nc = bacc.Bacc(None)
    x_ext = nc.declare_dram_parameter("x", [T, D], f16, isOutput=False)
    # xnT[dh, c*2+h, t] = xn[c*128+t, h*128+dh]  (host pre-transposed)
    xnt_ext = nc.declare_dram_parameter("xnt", [PB, NBLK * 2 * PB], f16,
                                        isOutput=False)
    # patterns [2, PB, 2PB], eye [PB, PB], and the contrarian selection
    # weights [PB(16), PB] packed into one tensor
    pe_ext = nc.declare_dram_parameter("pateye", [PB, 2 * 2 * PB + 2 * PB], f16,
                                       isOutput=False)
    consts_ext = nc.declare_dram_parameter("consts", [PB, 20], f32, isOutput=False)
    if not unit_affine:
        gain_ext = nc.declare_dram_parameter("gain_bc", [PB, D], f32, isOutput=False)
        bias_ext = nc.declare_dram_parameter("bias_bc", [PB, D], f32, isOutput=False)
    # out rows are in AGG_ORD (pipeline) order: rows [i*PB:(i+1)*PB] = tile AGG_ORD[i]
    out_ext = nc.declare_dram_parameter("out", [NTILE * PB, D], bf16, isOutput=True)

    with tile.TileContext(nc) as tc:
        with (
            tc.tile_pool(name="singles", bufs=1) as singles,
            tc.tile_pool(name="wp", bufs=3) as wp,
            tc.tile_pool(name="w1p", bufs=1) as w1p,
            tc.tile_pool(name="ap", bufs=3) as apool,
            tc.tile_pool(name="atp", bufs=8) as atpool,
            tc.tile_pool(name="small", bufs=8) as small,
            tc.tile_pool(name="bl", bufs=3) as blp,
            tc.tile_pool(name="ps_sim", bufs=2, space="PSUM") as ps_sim,
            tc.tile_pool(name="ps_ctx", bufs=3, space="PSUM") as ps_ctx,
        ):
            # ---- input DMAs, ordered by first use --------------------------
            xnt_sb = singles.tile([PB, NBLK, 2, PB], f16)
            xnt_re = xnt_ext[:].rearrange("p (c h q) -> p c h q", c=NBLK, h=2)
            nc.sync.dma_start(out=xnt_sb[:, 0:2, :, :], in_=xnt_re[:, 0:2, :, :])
            pe_sb = singles.tile([PB, 2 * 2 * PB + 2 * PB], f16)
            nc.scalar.dma_start(out=pe_sb, in_=pe_ext[:])
            pat_sb = pe_sb[:, 0:2 * 2 * PB].rearrange("p (q m) -> p q m", q=2)
            eye_sb = pe_sb[:, 2 * 2 * PB:2 * 2 * PB + PB]
            selw_sb = pe_sb[:, 2 * 2 * PB + PB:]
            consts_sb = singles.tile([PB, 20], f32)
            nc.scalar.dma_start(out=consts_sb, in_=consts_ext[:])
            nc.sync.dma_start(out=xnt_sb[:, 2:4, :, :], in_=xnt_re[:, 2:4, :, :])
            nc.sync.dma_start(out=xnt_sb[:, 4:NBLK, :, :], in_=xnt_re[:, 4:NBLK, :, :])
            x_all = singles.tile([PB, NBLK, D], f16)
            x_re = x_ext[:].rearrange("(c p) d -> p c d", p=PB)
            nc.sync.dma_start(out=x_all[:, 0:4, :], in_=x_re[:, 0:4, :])
            nc.sync.dma_start(out=x_all[:, 4:10, :], in_=x_re[:, 4:10, :])
            nc.sync.dma_start(out=x_all[:, 10:NBLK, :], in_=x_re[:, 10:NBLK, :])
            if not unit_affine:
                gain_sb = singles.tile([PB, D], f32)
                nc.sync.dma_start(out=gain_sb, in_=gain_ext[:])
                bias_sb = singles.tile([PB, D], f32)
                nc.sync.dma_start(out=bias_sb, in_=bias_ext[:])

            # first-touch copies: TensorScalar-family instructions encode only
            # one sync wait, so no TS op may be the first on its engine to
            # observe two DMA queues.  TensorCopy tolerates multiple waits.
            touch_b = singles.tile([PB, 4], f16)
            touch_f = singles.tile([PB, 2], f32)
            nc.vector.tensor_copy(touch_b[:, 0:1], xnt_sb[:, 0, 0, 0:1])
            nc.vector.tensor_copy(touch_b[:, 1:2], pe_sb[:, 0:1])
            nc.vector.tensor_copy(touch_b[:, 3:4], x_all[:, 0, 0:1])
            nc.vector.tensor_copy(touch_f[:, 0:1], consts_sb[:, 0:1])
            touch_p = singles.tile([PB, 3], f16)
            touch_pf = singles.tile([PB, 1], f32)
            nc.gpsimd.tensor_copy(touch_p[:, 0:1], x_all[:, 0, 0:1])
            nc.gpsimd.tensor_copy(touch_p[:, 1:2], pe_sb[:, 0:1])
            nc.gpsimd.tensor_copy(touch_p[:, 2:3], xnt_sb[:, 0, 0, 0:1])
            nc.gpsimd.tensor_copy(touch_pf, consts_sb[:, 0:1])
            touch_a = singles.tile([PB, 2], f32)
            nc.scalar.copy(touch_a[:, 0:1], consts_sb[:, 0:1])

            mix_ap = consts_sb[:, 0:1]
            alpha1m_ap = consts_sb[:, 2:3]      # alpha*(1-mix)
            scale_ap = consts_sb[:, 4:5]
            iota9_ap = consts_sb[:, 8:17]       # col 8+m holds m
            lane_ap = consts_sb[:, 17:18]       # p % 16

            # PE first-touch of eye (DMA queue) so real matmuls stay within
            # the fused-matmul wait budget


            # Gelu Act table load overlaps the input DMAs (Act is idle here)
            tbl = singles.tile([PB, 1], f32)
            nc.scalar.activation(tbl[:, 0:1], consts_sb[:, 0:1], Act.Gelu)

            # mix * eye (f16): lets the PE accumulate mix*x onto ctx in PSUM
            mixeye = singles.tile([PB, PB], f16)
            nc.vector.tensor_scalar_mul(mixeye, eye_sb, mix_ap)

            # index helpers for the contrarian chain (device-generated)
            iota_i = singles.tile([16, 512], mybir.dt.int32)
            nc.gpsimd.iota(iota_i, pattern=[[1, 512]], base=0,
                           channel_multiplier=0)
            iota_f = singles.tile([16, 512], f32)
            nc.gpsimd.tensor_copy(iota_f, iota_i)
            pcol_i = singles.tile([PB, 1], mybir.dt.int32)
            nc.gpsimd.iota(pcol_i, pattern=[[1, 1]], base=0,
                           channel_multiplier=16)
            pcol_f = singles.tile([PB, 1], f32)
            nc.gpsimd.tensor_copy(pcol_f, pcol_i)

            # contrarian scratch (one-shot tiles)
            spr_c = singles.tile([PB, 3, 16], f32)
            negs_c = singles.tile([PB, 3, 16], f32)
            l1v_c = singles.tile([PB, 3, 8], f32)
            l1i_c = singles.tile([PB, 3, 8], mybir.dt.uint16)
            g1f_c = singles.tile([PB, 3, 4], f32)
            pk16_c = singles.tile([PB, PB], mybir.dt.uint16)
            nc.gpsimd.memzero(pk16_c)
            pkT_c = singles.tile([PB, 1, PB], mybir.dt.uint16)
            L2r_c = singles.tile([16, 1024], mybir.dt.uint16)
            c2v_c = singles.tile([16, 8], f32)
            c2i_c = singles.tile([16, 8], mybir.dt.uint16)
            c2f_c = singles.tile([16, 3], f32)
            mt_c = singles.tile([16, 512], f32)
            gidxf_c = singles.tile([16, 3], f32)
            g9_c = singles.tile([1, 9], f32)
            bc_c = singles.tile([PB, 9], f32)
            mt9_c = singles.tile([PB, 9], f32)
            g1d_c = singles.tile([PB, 1], f32)
            gidx9_c = singles.tile([PB, 2], mybir.dt.int16)
            nc.gpsimd.memzero(gidx9_c)
            gath_c = singles.tile([PB, 1, D], f16)
            gather_sem = nc.alloc_semaphore("gather_done")

            # ---- per-tile pipeline stages --------------------------------
            state = {}

            def stage_sim(k):
                nb = WB[k]
                W = nb * PB
                own = OWN[k]
                pool = w1p if k == 1 else wp
                w_t = pool.tile([PB, W], f32, tag="w1" if k == 1 else "w")
                nchunk = (W + 1023) // 1024
                vall = small.tile([PB, 2, 8], f32, tag="vall")
                for j in range(nchunk):
                    lo = j * 1024
                    n = min(1024, W - lo)
                    ps = ps_sim.tile([PB, n], f32, tag="ps_sim")
                    last_chunk = (j == nchunk - 1)
                    for s in range(0, n, 512):
                        m = min(512, n - s)
                        cb, ncb = (lo + s) // PB, m // PB
                        last_sub = (s + m == n)
                        nc.tensor.matmul(
                            ps[:, s:s + m], xnt_sb[:, own, 0, :],
                            xnt_sb[:, cb:cb + ncb, 0, :],
                            start=True, stop=False)
                        if last_chunk and last_sub:
                            # PE adds the causal penalty pattern onto the
                            # tail: eye^T @ pattern == pattern
                            nc.tensor.matmul(
                                ps[:, n - 256:n], eye_sb, pat_sb[:, k % 2, :],
                                start=False, stop=False, skip_group_check=True)
                        nc.tensor.matmul(
                            ps[:, s:s + m], xnt_sb[:, own, 1, :],
                            xnt_sb[:, cb:cb + ncb, 1, :],
                            start=False, stop=True)
                    # psum -> w on Act; chunk top-8 on DVE straight from PSUM
                    nc.scalar.copy(w_t[:, lo:lo + n], ps)
                    nc.vector.max(out=vall[:, j, :], in_=ps)
                state[k] = {"w": w_t, "vall": vall, "nchunk": nchunk}

            def stage_select(k):
                nb = WB[k]
                W = nb * PB
                st = state[k]
                w_t = st["w"]
                vall = st["vall"]
                if st["nchunk"] > 1:
                    v8 = small.tile([PB, 8], f32, tag="v8")
                    nc.vector.max(out=v8, in_=vall)
                else:
                    v8 = vall[:, 0, :]
                tau = small.tile([PB, 1], f32, tag="tau")
                nc.vector.tensor_scalar_max(tau, v8[:, 7:8], -1e4)
                # deg + 1e-6: keeps recip finite for the all-masked row 0
                # (whose A is empty anyway since tau is clamped to -1e4)
                cnt8 = small.tile([PB, 8], f32, tag="cnt8")
                deg = small.tile([PB, 1], f32, tag="deg")
                nc.vector.tensor_scalar(cnt8, v8, -1e4, 1.25e-7, op0=Alu.is_gt,
                                        op1=Alu.add, accum_out=deg)
                coef = small.tile([PB, 1], f32, tag="coef")
                nc.vector.reciprocal(coef, deg)

                # A = (w >= tau) / deg, f16; the alpha*(1-mix) factor is
                # folded into the host-scaled x_all
                A_t = apool.tile([PB, W], f16, tag="A")
                nc.gpsimd.memzero(A_t[:, 0:2])
                nc.gpsimd.tensor_scalar(A_t, w_t, tau, coef,
                                        op0=Alu.is_ge, op1=Alu.mult)

                # batched xbar transpose: AT[s, c, t] = A[t, c*128+s]
                # (contiguous destination tile -- see module docstring)
                at_t = atpool.tile([PB, nb, PB], f16, tag="AT")
                nc.sync.dma_start_transpose(at_t, A_t)
                st["AT"] = at_t

                # ---- contrarian chain (tile 1 only): exact bottom-k of the
                # last 3 rows (global rows 2045..2047, partitions 125..127 on
                # even-parity cores).  The row's candidates are spread across
                # partitions, min-selected hierarchically, their global
                # indices recovered by iota-match, and the picked x rows
                # gathered by DMA; a tiny K=16 matmul with host-shipped
                # static weights (zero on odd cores) adds the result to
                # tile 1's ctx PSUM.  Everything here is off the main spine.
                if k == 1:
                    # level 1: spread each row's 2048 candidates to [128, 16]
                    for r in range(3):
                        nc.sync.dma_start(out=spr_c[:, r, :],
                                          in_=w_t[125 + r:126 + r, :])
                    nc.vector.tensor_scalar_mul(negs_c, spr_c, -1.0)
                    for r in range(3):
                        nc.vector.max(out=l1v_c[:, r, :], in_=negs_c[:, r, :])
                        nc.vector.max_index(l1i_c[:, r, :], l1v_c[:, r, :],
                                            negs_c[:, r, :])
                    # globalize indices (s = p*16 + j), pack values (f16)
                    # and indices (u16) into one 2-byte tile, and move both
                    # partition->free with a single cheap xbar transpose:
                    # plane r*8+s (s<4: values, s>=4: indices) lands on
                    # partition r*8+s; L2 row r = [v|g] each 512 wide with
                    # flat position q = s*128 + p
                    nc.vector.tensor_scalar(g1f_c, l1i_c[:, :, 0:4], pcol_f,
                                            None, op0=Alu.add)
                    pk_re = pk16_c[:, 0:24].rearrange("p (r s) -> p r s", r=3)
                    nc.vector.tensor_copy(pk_re[:, :, 0:4].bitcast(f16),
                                          l1v_c[:, :, 0:4])
                    nc.vector.tensor_copy(pk_re[:, :, 4:8], g1f_c)
                    nc.sync.dma_start_transpose(pkT_c, pk16_c)
                    for r in range(3):
                        nc.sync.dma_start(out=L2r_c[r:r + 1, :],
                                          in_=pkT_c[8 * r:8 * r + 8, 0, :])
                    L2v = L2r_c[0:3, 0:512].bitcast(f16)
                    L2g = L2r_c[0:3, 512:1024]
                    nc.vector.max(out=c2v_c[0:3, :], in_=L2v)
                    nc.vector.max_index(c2i_c[0:3, :], c2v_c[0:3, :], L2v)
                    # picks are at descending positions 1..3 (the 3-r
                    # sentinels occupy positions 0..2-r); recover global
                    # indices by exact iota-match against gcmp
                    nc.vector.tensor_copy(c2f_c[0:3, :], c2i_c[0:3, 1:4])
                    for j in range(3):
                        nc.vector.scalar_tensor_tensor(
                            mt_c[0:3, :], iota_f[0:3, :], c2f_c[0:3, j:j + 1],
                            L2g, op0=Alu.is_equal, op1=Alu.mult,
                            accum_out=gidxf_c[0:3, j:j + 1])
                    # slot layout: partition r*3+j holds pick j of row r.
                    # On hardware each of the 8 gpsimd cores reads the idx
                    # list from ITS OWN 16-partition group, so replicate:
                    # consolidate to one row, broadcast to all partitions,
                    # then each partition diagonal-selects its lane's slot.
                    idma = nc.sync.dma_start(out=g9_c,
                                             in_=gidxf_c[0:3, 0:3])
                    nc.gpsimd.partition_broadcast(bc_c, g9_c)
                    nc.vector.scalar_tensor_tensor(
                        mt9_c, iota9_ap, lane_ap, bc_c,
                        op0=Alu.is_equal, op1=Alu.mult, accum_out=g1d_c)
                    icpy = nc.vector.tensor_copy(gidx9_c[:, 0:1], g1d_c)
                    gth = nc.gpsimd.dma_gather(gath_c, x_ext[:],
                                               gidx9_c[:, 0:1],
                                               num_idxs=16, num_idxs_reg=16,
                                               elem_size=D)
                    # SWDGE gathers are not auto-sequenced by the tile
                    # framework: order idx-write -> gather -> consumer, and
                    # signal transfer completion through an explicit sem
                    # (the Pool-engine sem only covers descriptor gen).
                    add_dep_helper(gth.ins, idma.ins,
                                   reason="gather waits for index path")
                    add_dep_helper(gth.ins, icpy.ins,
                                   reason="gather waits for index write")
                    gth.then_inc(gather_sem, 16)
                    st["gather"] = gth
                del st["w"]

            def stage_agg(k, pos):
                nb = WB[k]
                own = OWN[k]
                st = state[k]
                at_t = st["AT"]

                # ctx accumulation; the mixeye matmul folds the mix*x blend
                # term into the same PSUM group (ctx coefs carry (1-mix)).
                ctx_ps = ps_ctx.tile([PB, D], f32, tag="ctx")
                nc.tensor.matmul(ctx_ps, mixeye, x_all[:, own, :],
                                 start=True, stop=False)
                for c in range(nb):
                    nc.tensor.matmul(ctx_ps, at_t[:, c, :],
                                     x_all[:, c, :], start=False,
                                     stop=(c == nb - 1 and k != 1))
                if k == 1:
                    # contrarian contribution: gathered x rows weighted by
                    # the static selection matrix (coefc at (slot, 125+r))
                    nc.tensor.wait_ge(gather_sem, 16)
                    mm = nc.tensor.matmul(ctx_ps, selw_sb[0:16, :],
                                          gath_c[0:16, 0, :],
                                          start=False, stop=True)
                    add_dep_helper(mm.ins, st["gather"].ins,
                                   reason="sel matmul waits for gather")

                # gelu straight off PSUM (Act) into the bf16 out buffer;
                # the final *scale is applied on the host
                if not unit_affine:
                    z_t = blp.tile([PB, D], f32, tag="z")
                    nc.vector.tensor_mul(z_t, ctx_ps, gain_sb)
                    nc.vector.tensor_add(z_t, z_t, bias_sb)
                    gelu_src = z_t
                else:
                    gelu_src = ctx_ps
                if pos >= 6:
                    # last two tiles ship solo to shorten the pipeline tail
                    d_t = blp.tile([PB, 1, D], bf16, tag="pair", name="d_t")
                    nc.scalar.activation(d_t[:, 0, :], gelu_src, Act.Gelu)
                    nc.sync.dma_start(out=out_ext[pos * PB:(pos + 1) * PB, :],
                                      in_=d_t[:, 0, :])
                else:
                    half = pos % 2
                    if half == 0:
                        pair_t = blp.tile([PB, 2, D], bf16, tag="pair",
                                          name="pair_t")
                        state["pair"] = pair_t
                    d_t = state["pair"]
                    nc.scalar.activation(d_t[:, half, :], gelu_src, Act.Gelu)
                    if half == 1:
                        lo = (pos - 1) * PB
                        nc.sync.dma_start(
                            out=out_ext[lo:lo + 2 * PB, :].rearrange(
                                "(c p) d -> p c d", p=PB),
                            in_=d_t)
                del state[k]

            # emission: all sims first (the in-order PE queue would
            # head-of-line block on any agg whose xbar transpose is still
            # in flight), selects interleaved, then all aggs
            for i in range(len(TORD)):
                stage_sim(TORD[i])
                if i >= 1:
                    stage_select(TORD[i - 1])
            stage_select(TORD[-1])
            for j in range(len(AGG_ORD)):
                stage_agg(AGG_ORD[j], j)

    nc.compile()
    return nc


def _get_program(unit_affine=True):
    key = bool(unit_affine)
    if key not in _PROGRAMS:
        _PROGRAMS[key] = _build_program(unit_affine=key)
    return _PROGRAMS[key]


def _make_in_maps(inputs):
    """Host-side prep: returns (in_maps for cores 0-7, unit_affine flag)."""
    x = np.asarray(inputs["x"], dtype=np.float32)
    gain = np.asarray(inputs["gain"], dtype=np.float32).reshape(D)
    bias = np.asarray(inputs["bias"], dtype=np.float32).reshape(D)
    log_mix = float(np.asarray(inputs["log_mix"]))
    log_alpha = float(np.asarray(inputs["log_alpha"]))
    log_scale = float(np.asarray(inputs["log_scale"]))

    mix = np.float32(1.0 / (1.0 + np.exp(-np.float64(log_mix))))
    alpha = np.float32(1.0 / (1.0 + np.exp(-np.float64(log_alpha))))
    scale = np.float32(np.logaddexp(0.0, np.float64(log_scale)) + 0.01)
    unit_affine = bool(np.all(gain == 1.0) and np.all(bias == 0.0))

    alpha1m = alpha * (np.float32(1.0) - mix)   # folded into shipped x
    consts = np.zeros((PB, 20), np.float32)
    consts[:, 0] = mix / alpha1m                # mixeye coefficient
    consts[:, 1] = np.float32(1.0) - mix
    consts[:, 2] = alpha * (np.float32(1.0) - mix)
    consts[:, 3] = (np.float32(1.0) - alpha) * (np.float32(1.0) - mix)
    consts[:, 4] = scale
    consts[:, 8:17] = np.arange(9, dtype=np.float32)[None, :]
    consts[:, 17] = (np.arange(PB) % 16).astype(np.float32)
    eye_bf = np.eye(PB, dtype=np.float32).astype(np.float16)
    patterns = _build_patterns()

    swap_perm = np.arange(NBLK).reshape(-1, 2)[:, ::-1].reshape(-1)

    # host-side normalize (f32) once per batch
    xn = x / np.maximum(np.linalg.norm(x, axis=-1, keepdims=True), 1e-12)

    in_maps = []
    for c in range(8):
        b, p = c // 2, c % 2
        xb = x[b]
        xnb = xn[b]
        if p:
            xb = xb.reshape(NBLK, PB, D)[swap_perm].reshape(T, D)
            xnb = xnb.reshape(NBLK, PB, D)[swap_perm].reshape(T, D)
        # xnT layout [dh, c, h, t]
        xnt = np.ascontiguousarray(
            xnb.reshape(NBLK, PB, 2, PB).transpose(3, 0, 2, 1)
            .reshape(PB, NBLK * 2 * PB).astype(np.float16))
        # contrarian selection weights: slot r*3+j (partition) -> column
        # 125+r gets (1-alpha)(1-mix)/(r+1) iff pick position j >= 2-r;
        # only even-parity cores own global rows 2045..2047
        selw = np.zeros((PB, PB), np.float32)
        if p == 0:
            for r in range(3):
                for j in range(3):
                    if j >= 2 - r:
                        selw[r * 3 + j, 125 + r] = (
                            (np.float32(1.0) - alpha)
                            * (np.float32(1.0) - mix)
                            / (np.float32(r + 1) * alpha1m))
        pateye = np.concatenate(
            [patterns[p].astype(np.float16).transpose(1, 0, 2).reshape(PB, -1),
             eye_bf, selw.astype(np.float16)], axis=1)
        im = {
            "x": np.ascontiguousarray((xb * alpha1m).astype(np.float16)),
            "xnt": xnt,
            "pateye": np.ascontiguousarray(pateye),
            "consts": consts,
        }
        if not unit_affine:
            im["gain_bc"] = np.ascontiguousarray(
                np.broadcast_to(gain[None, :], (PB, D)).astype(np.float32))
            im["bias_bc"] = np.ascontiguousarray(
                np.broadcast_to(bias[None, :], (PB, D)).astype(np.float32))
        in_maps.append(im)
    return in_maps, unit_affine


def kernel(**inputs):
    log_scale = float(np.asarray(inputs["log_scale"]))
    scale = np.float32(np.logaddexp(0.0, np.float64(log_scale)) + 0.01)
    in_maps, unit_affine = _make_in_maps(inputs)
    from concourse.bass_utils import run_bass_kernel_spmd
    nc = _get_program(unit_affine)
    res = run_bass_kernel_spmd(nc, in_maps, list(range(8))).results

    out = np.empty((B, T, D), np.float32)
    for c in range(8):
        b, p = c // 2, c % 2
        o = np.asarray(res[c]["out"]).astype(np.float32) * scale
        for i in range(NTILE):
            k = AGG_ORD[i]
            g_act = OWN[k] ^ p
            out[b, g_act * PB:(g_act + 1) * PB, :] = o[i * PB:(i + 1) * PB, :]
    return out


# revision 63
# speedup vs baseline: 1.0032x; 1.0032x over previous
"""Trainium2 Bass kernel for nn_DGN4 (gnn_message_passing)  -- v3.

Reference semantics (B=4, T=2048, D=256, K_SIM=8, K_CON=4):
  xn    = x / max(||x||, 1e-12)                       (row L2-normalize)
  sim   = xn @ xn^T, causally masked (strictly past), masked = -1e9
  A_sim = top-8 per row (one-hot), zeroed outside past
  A_con = "bottom-4" of sim excluding A_sim -- future columns score +1e9
          in the negated space, so only rows T-3..T-1 get 1..3 real
          con-neighbors; everywhere else A_con == 0.
  msg_* = degree-normalized mean of selected x rows
  ctx   = alpha*msg_pos + (1-alpha)*msg_neg
  delta = gelu(mix*x + (1-mix)*ctx) * scale   (exact erf gelu)

Sharding: 8 cores = 4 batches x 2 row-shards, one uniform SPMD program;
per-core differences are data only (odd cores get adjacent 128-row blocks
swapped so the same static tile offsets address their rows).

v3 structure (vs v2):
  - xn and its transpose are precomputed on the host and shipped as xnT
    (f16): the on-device norm/normalize/PE-transpose/psum-copy prologue
    is gone.
  - A^T is produced by ONE batched DMA-engine xbar transpose per tile
    (3D contiguous out => per-128-block transposes); PE transposes and
    the psum->sbuf copies of A^T are gone.  NOTE: the xbar destination
    MUST be a contiguous tile -- a padded/sliced dst is silently wrong
    on hardware and breaks dependency tracking.
  - top-8 runs chunk-wise on DVE directly from PSUM, in parallel with
    the Act psum->w copies, so selection overlaps the sim matmuls.
  - outputs are bf16, written in pipeline order and DMA'd in pairs.
  - selection thresholds stay fp32 (f16 sim values would tie at the
    8th-largest and break the degree normalization).
"""

import numpy as np

B, T, D = 4, 2048, 256
PB = 128                 # partition block
NBLK = T // PB           # 16 row/col blocks per batch
NTILE = 8                # program tiles per core
# width (in 128-blocks) and own-block index per program tile; widths pair to 18
WB = [2, 16, 4, 14, 6, 12, 8, 10]
OWN = [0, 15, 2, 13, 4, 11, 6, 9]
NEGF = -60000.0

# tile processing order (pipeline): tile 0/2 need only the first x blocks
# (earliest PE start), tile 1 (full width + contrarian chain) early so its
# long select pipeline overlaps the rest.
TORD = [0, 2, 1, 3, 5, 7, 6, 4]
# agg emission order: tile 1 (whose contrarian gather the PE must wait on)
# goes last so the in-order PE queue never stalls mid-pipeline
AGG_ORD = [0, 2, 3, 5, 7, 6, 4, 1]

_PROGRAMS = {}


def _build_patterns():
    """Penalty patterns (0 = keep, NEGF = masked) for the last two 128-col
    blocks of each program tile, as a function of tile parity (k%2) and
    core parity.  patterns[parity] is [2, PB, 2*PB] float32."""
    tri = np.where(np.arange(PB)[None, :] < np.arange(PB)[:, None], 0.0, NEGF)
    keep = np.zeros((PB, PB), np.float32)
    mask = np.full((PB, PB), NEGF, np.float32)
    out = []
    for parity in (0, 1):
        m = np.zeros((2, PB, 2 * PB), np.float32)
        if parity == 0:
            m[0] = np.concatenate([tri, mask], axis=1)   # even k
            m[1] = np.concatenate([keep, tri], axis=1)   # odd k
        else:
            m[0] = np.concatenate([tri, keep], axis=1)
            m[1] = np.concatenate([mask, tri], axis=1)
        out.append(m.astype(np.float32))
    return out


def _build_program(unit_affine=True):
    import concourse.bacc as bacc
    import concourse.tile as tile
    from concourse import mybir
    from concourse.tile_rust import add_dep_helper

    f32 = mybir.dt.float32
    f16 = mybir.dt.float16
    bf16 = mybir.dt.bfloat16
    Alu = mybir.AluOpType
    Act = mybir.ActivationFunctionType

    nc = bacc.Bacc(None)
    x_ext = nc.declare_dram_parameter("x", [T, D], f16, isOutput=False)
    # xnT[dh, c*2+h, t] = xn[c*128+t, h*128+dh]  (host pre-transposed)
    xnt_ext = nc.declare_dram_parameter("xnt", [PB, NBLK * 2 * PB], f16,
                                        isOutput=False)
    # patterns [2, PB, 2PB], eye [PB, PB], and the contrarian selection
    # weights [PB(16), PB] packed into one tensor
    pe_ext = nc.declare_dram_parameter("pateye", [PB, 2 * 2 * PB + 2 * PB], f16,
                                       isOutput=False)
    consts_ext = nc.declare_dram_parameter("consts", [PB, 20], f32, isOutput=False)
    if not unit_affine:
        gain_ext = nc.declare_dram_parameter("gain_bc", [PB, D], f32, isOutput=False)
        bias_ext = nc.declare_dram_parameter("bias_bc", [PB, D], f32, isOutput=False)
    # out rows are in AGG_ORD (pipeline) order: rows [i*PB:(i+1)*PB] = tile AGG_ORD[i]
    out_ext = nc.declare_dram_parameter("out", [NTILE * PB, D], bf16, isOutput=True)

    with tile.TileContext(nc) as tc:
        with (
            tc.tile_pool(name="singles", bufs=1) as singles,
            tc.tile_pool(name="wp", bufs=3) as wp,
            tc.tile_pool(name="w1p", bufs=1) as w1p,
            tc.tile_pool(name="ap", bufs=3) as apool,
            tc.tile_pool(name="atp", bufs=8) as atpool,
            tc.tile_pool(name="small", bufs=8) as small,
            tc.tile_pool(name="bl", bufs=3) as blp,
            tc.tile_pool(name="ps_sim", bufs=2, space="PSUM") as ps_sim,
            tc.tile_pool(name="ps_ctx", bufs=3, space="PSUM") as ps_ctx,
        ):
            # ---- input DMAs, ordered by first use --------------------------
            xnt_sb = singles.tile([PB, NBLK, 2, PB], f16)
            xnt_re = xnt_ext[:].rearrange("p (c h q) -> p c h q", c=NBLK, h=2)
            nc.sync.dma_start(out=xnt_sb[:, 0:2, :, :], in_=xnt_re[:, 0:2, :, :])
            pe_sb = singles.tile([PB, 2 * 2 * PB + 2 * PB], f16)
            nc.scalar.dma_start(out=pe_sb, in_=pe_ext[:])
            pat_sb = pe_sb[:, 0:2 * 2 * PB].rearrange("p (q m) -> p q m", q=2)
            eye_sb = pe_sb[:, 2 * 2 * PB:2 * 2 * PB + PB]
            selw_sb = pe_sb[:, 2 * 2 * PB + PB:]
            consts_sb = singles.tile([PB, 20], f32)
            nc.scalar.dma_start(out=consts_sb, in_=consts_ext[:])
            nc.sync.dma_start(out=xnt_sb[:, 2:4, :, :], in_=xnt_re[:, 2:4, :, :])
            nc.sync.dma_start(out=xnt_sb[:, 4:NBLK, :, :], in_=xnt_re[:, 4:NBLK, :, :])
            x_all = singles.tile([PB, NBLK, D], f16)
            x_re = x_ext[:].rearrange("(c p) d -> p c d", p=PB)
            nc.sync.dma_start(out=x_all[:, 0:4, :], in_=x_re[:, 0:4, :])
            nc.sync.dma_start(out=x_all[:, 4:10, :], in_=x_re[:, 4:10, :])
            nc.sync.dma_start(out=x_all[:, 10:NBLK, :], in_=x_re[:, 10:NBLK, :])
            if not unit_affine:
                gain_sb = singles.tile([PB, D], f32)
                nc.sync.dma_start(out=gain_sb, in_=gain_ext[:])
                bias_sb = singles.tile([PB, D], f32)
                nc.sync.dma_start(out=bias_sb, in_=bias_ext[:])

            # first-touch copies: TensorScalar-family instructions encode only
            # one sync wait, so no TS op may be the first on its engine to
            # observe two DMA queues.  TensorCopy tolerates multiple waits.
            touch_b = singles.tile([PB, 4], f16)
            touch_f = singles.tile([PB, 2], f32)
            nc.vector.tensor_copy(touch_b[:, 0:1], xnt_sb[:, 0, 0, 0:1])
            nc.vector.tensor_copy(touch_b[:, 1:2], pe_sb[:, 0:1])
            nc.vector.tensor_copy(touch_b[:, 3:4], x_all[:, 0, 0:1])
            nc.vector.tensor_copy(touch_f[:, 0:1], consts_sb[:, 0:1])
            touch_p = singles.tile([PB, 3], f16)
            touch_pf = singles.tile([PB, 1], f32)
            nc.gpsimd.tensor_copy(touch_p[:, 0:1], x_all[:, 0, 0:1])
            nc.gpsimd.tensor_copy(touch_p[:, 1:2], pe_sb[:, 0:1])
            nc.gpsimd.tensor_copy(touch_p[:, 2:3], xnt_sb[:, 0, 0, 0:1])
            nc.gpsimd.tensor_copy(touch_pf, consts_sb[:, 0:1])
            touch_a = singles.tile([PB, 2], f32)
            nc.scalar.copy(touch_a[:, 0:1], consts_sb[:, 0:1])

            mix_ap = consts_sb[:, 0:1]
            alpha1m_ap = consts_sb[:, 2:3]      # alpha*(1-mix)
            scale_ap = consts_sb[:, 4:5]
            iota9_ap = consts_sb[:, 8:17]       # col 8+m holds m
            lane_ap = consts_sb[:, 17:18]       # p % 16

            # PE first-touch of eye (DMA queue) so real matmuls stay within
            # the fused-matmul wait budget


            # Gelu Act table load overlaps the input DMAs (Act is idle here)
            tbl = singles.tile([PB, 1], f32)
            nc.scalar.activation(tbl[:, 0:1], consts_sb[:, 0:1], Act.Gelu)

            # mix * eye (f16): lets the PE accumulate mix*x onto ctx in PSUM
            mixeye = singles.tile([PB, PB], f16)
            nc.vector.tensor_scalar_mul(mixeye, eye_sb, mix_ap)

            # index helpers for the contrarian chain (device-generated)
            iota_i = singles.tile([16, 512], mybir.dt.int32)
            nc.gpsimd.iota(iota_i, pattern=[[1, 512]], base=0,
                           channel_multiplier=0)
            iota_f = singles.tile([16, 512], f32)
            nc.gpsimd.tensor_copy(iota_f, iota_i)
            pcol_i = singles.tile([PB, 1], mybir.dt.int32)
            nc.gpsimd.iota(pcol_i, pattern=[[1, 1]], base=0,
                           channel_multiplier=16)
            pcol_f = singles.tile([PB, 1], f32)
            nc.gpsimd.tensor_copy(pcol_f, pcol_i)

            # contrarian scratch (one-shot tiles)
            spr_c = singles.tile([PB, 3, 16], f32)
            negs_c = singles.tile([PB, 3, 16], f32)
            l1v_c = singles.tile([PB, 3, 8], f32)
            l1i_c = singles.tile([PB, 3, 8], mybir.dt.uint16)
            g1f_c = singles.tile([PB, 3, 4], f32)
            pk16_c = singles.tile([PB, PB], mybir.dt.uint16)
            nc.gpsimd.memzero(pk16_c)
            pkT_c = singles.tile([PB, 1, PB], mybir.dt.uint16)
            L2r_c = singles.tile([16, 1024], mybir.dt.uint16)
            c2v_c = singles.tile([16, 8], f32)
            c2i_c = singles.tile([16, 8], mybir.dt.uint16)
            c2f_c = singles.tile([16, 3], f32)
            mt_c = singles.tile([16, 512], f32)
            gidxf_c = singles.tile([16, 3], f32)
            g9_c = singles.tile([1, 9], f32)
            bc_c = singles.tile([PB, 9], f32)
            mt9_c = singles.tile([PB, 9], f32)
            g1d_c = singles.tile([PB, 1], f32)
            gidx9_c = singles.tile([PB, 2], mybir.dt.int16)
            nc.gpsimd.memzero(gidx9_c)
            gath_c = singles.tile([PB, 1, D], f16)
            gather_sem = nc.alloc_semaphore("gather_done")

            # ---- per-tile pipeline stages --------------------------------
            state = {}

            def stage_sim(k):
                nb = WB[k]
                W = nb * PB
                own = OWN[k]
                pool = w1p if k == 1 else wp
                w_t = pool.tile([PB, W], f32, tag="w1" if k == 1 else "w")
                nchunk = (W + 1023) // 1024
                vall = small.tile([PB, 2, 8], f32, tag="vall")
                for j in range(nchunk):
                    lo = j * 1024
                    n = min(1024, W - lo)
                    ps = ps_sim.tile([PB, n], f32, tag="ps_sim")
                    last_chunk = (j == nchunk - 1)
                    for s in range(0, n, 512):
                        m = min(512, n - s)
                        cb, ncb = (lo + s) // PB, m // PB
                        last_sub = (s + m == n)
                        nc.tensor.matmul(
                            ps[:, s:s + m], xnt_sb[:, own, 0, :],
                            xnt_sb[:, cb:cb + ncb, 0, :],
                            start=True, stop=False)
                        if last_chunk and last_sub:
                            # PE adds the causal penalty pattern onto the
                            # tail: eye^T @ pattern == pattern
                            nc.tensor.matmul(
                                ps[:, n - 256:n], eye_sb, pat_sb[:, k % 2, :],
                                start=False, stop=False, skip_group_check=True)
                        nc.tensor.matmul(
                            ps[:, s:s + m], xnt_sb[:, own, 1, :],
                            xnt_sb[:, cb:cb + ncb, 1, :],
                            start=False, stop=True)
                    # psum -> w on Act; chunk top-8 on DVE straight from PSUM
                    nc.scalar.copy(w_t[:, lo:lo + n], ps)
                    nc.vector.max(out=vall[:, j, :], in_=ps)
                state[k] = {"w": w_t, "vall": vall, "nchunk": nchunk}

            def stage_select(k):
                nb = WB[k]
                W = nb * PB
                st = state[k]
                w_t = st["w"]
                vall = st["vall"]
                if st["nchunk"] > 1:
                    v8 = small.tile([PB, 8], f32, tag="v8")
                    nc.vector.max(out=v8, in_=vall)
                else:
                    v8 = vall[:, 0, :]
                tau = small.tile([PB, 1], f32, tag="tau")
                nc.vector.tensor_scalar_max(tau, v8[:, 7:8], -1e4)
                # deg + 1e-6: keeps recip finite for the all-masked row 0
                # (whose A is empty anyway since tau is clamped to -1e4)
                cnt8 = small.tile([PB, 8], f32, tag="cnt8")
                deg = small.tile([PB, 1], f32, tag="deg")
                nc.vector.tensor_scalar(cnt8, v8, -1e4, 1.25e-7, op0=Alu.is_gt,
                                        op1=Alu.add, accum_out=deg)
                coef = small.tile([PB, 1], f32, tag="coef")
                nc.vector.reciprocal(coef, deg)

                # A = (w >= tau) / deg, f16; the alpha*(1-mix) factor is
                # folded into the host-scaled x_all
                A_t = apool.tile([PB, W], f16, tag="A")
                nc.gpsimd.memzero(A_t[:, 0:2])
                nc.gpsimd.tensor_scalar(A_t, w_t, tau, coef,
                                        op0=Alu.is_ge, op1=Alu.mult)

                # batched xbar transpose: AT[s, c, t] = A[t, c*128+s]
                # (contiguous destination tile -- see module docstring)
                at_t = atpool.tile([PB, nb, PB], f16, tag="AT")
                nc.sync.dma_start_transpose(at_t, A_t)
                st["AT"] = at_t

                # ---- contrarian chain (tile 1 only): exact bottom-k of the
                # last 3 rows (global rows 2045..2047, partitions 125..127 on
                # even-parity cores).  The row's candidates are spread across
                # partitions, min-selected hierarchically, their global
                # indices recovered by iota-match, and the picked x rows
                # gathered by DMA; a tiny K=16 matmul with host-shipped
                # static weights (zero on odd cores) adds the result to
                # tile 1's ctx PSUM.  Everything here is off the main spine.
                if k == 1:
                    # level 1: spread each row's 2048 candidates to [128, 16]
                    for r in range(3):
                        nc.sync.dma_start(out=spr_c[:, r, :],
                                          in_=w_t[125 + r:126 + r, :])
                    nc.vector.tensor_scalar_mul(negs_c, spr_c, -1.0)
                    for r in range(3):
                        nc.vector.max(out=l1v_c[:, r, :], in_=negs_c[:, r, :])
                        nc.vector.max_index(l1i_c[:, r, :], l1v_c[:, r, :],
                                            negs_c[:, r, :])
                    # globalize indices (s = p*16 + j), pack values (f16)
                    # and indices (u16) into one 2-byte tile, and move both
                    # partition->free with a single cheap xbar transpose:
                    # plane r*8+s (s<4: values, s>=4: indices) lands on
                    # partition r*8+s; L2 row r = [v|g] each 512 wide with
                    # flat position q = s*128 + p
                    nc.vector.tensor_scalar(g1f_c, l1i_c[:, :, 0:4], pcol_f,
                                            None, op0=Alu.add)
                    pk_re = pk16_c[:, 0:24].rearrange("p (r s) -> p r s", r=3)
                    nc.vector.tensor_copy(pk_re[:, :, 0:4].bitcast(f16),
                                          l1v_c[:, :, 0:4])
                    nc.vector.tensor_copy(pk_re[:, :, 4:8], g1f_c)
                    nc.sync.dma_start_transpose(pkT_c, pk16_c)
                    for r in range(3):
                        nc.sync.dma_start(out=L2r_c[r:r + 1, :],
                                          in_=pkT_c[8 * r:8 * r + 8, 0, :])
                    L2v = L2r_c[0:3, 0:512].bitcast(f16)
                    L2g = L2r_c[0:3, 512:1024]
                    nc.vector.max(out=c2v_c[0:3, :], in_=L2v)
                    nc.vector.max_index(c2i_c[0:3, :], c2v_c[0:3, :], L2v)
                    # picks are at descending positions 1..3 (the 3-r
                    # sentinels occupy positions 0..2-r); recover global
                    # indices by exact iota-match against gcmp
                    nc.vector.tensor_copy(c2f_c[0:3, :], c2i_c[0:3, 1:4])
                    for j in range(3):
                        nc.vector.scalar_tensor_tensor(
                            mt_c[0:3, :], iota_f[0:3, :], c2f_c[0:3, j:j + 1],
                            L2g, op0=Alu.is_equal, op1=Alu.mult,
                            accum_out=gidxf_c[0:3, j:j + 1])
                    # slot layout: partition r*3+j holds pick j of row r.
                    # On hardware each of the 8 gpsimd cores reads the idx
                    # list from ITS OWN 16-partition group, so replicate:
                    # consolidate to one row, broadcast to all partitions,
                    # then each partition diagonal-selects its lane's slot.
                    idma = nc.sync.dma_start(out=g9_c,
                                             in_=gidxf_c[0:3, 0:3])
                    nc.gpsimd.partition_broadcast(bc_c, g9_c)
                    nc.vector.scalar_tensor_tensor(
                        mt9_c, iota9_ap, lane_ap, bc_c,
                        op0=Alu.is_equal, op1=Alu.mult, accum_out=g1d_c)
                    icpy = nc.vector.tensor_copy(gidx9_c[:, 0:1], g1d_c)
                    gth = nc.gpsimd.dma_gather(gath_c, x_ext[:],
                                               gidx9_c[:, 0:1],
                                               num_idxs=16, num_idxs_reg=16,
                                               elem_size=D)
                    # SWDGE gathers are not auto-sequenced by the tile
                    # framework: order idx-write -> gather -> consumer, and
                    # signal transfer completion through an explicit sem
                    # (the Pool-engine sem only covers descriptor gen).
                    add_dep_helper(gth.ins, idma.ins,
                                   reason="gather waits for index path")
                    add_dep_helper(gth.ins, icpy.ins,
                                   reason="gather waits for index write")
                    gth.then_inc(gather_sem, 16)
                    st["gather"] = gth
                del st["w"]

            def stage_agg(k, pos):
                nb = WB[k]
                own = OWN[k]
                st = state[k]
                at_t = st["AT"]

                # ctx accumulation; the mixeye matmul folds the mix*x blend
                # term into the same PSUM group (ctx coefs carry (1-mix)).
                ctx_ps = ps_ctx.tile([PB, D], f32, tag="ctx")
                nc.tensor.matmul(ctx_ps, mixeye, x_all[:, own, :],
                                 start=True, stop=False)
                for c in range(nb):
                    nc.tensor.matmul(ctx_ps, at_t[:, c, :],
                                     x_all[:, c, :], start=False,
                                     stop=(c == nb - 1 and k != 1))
                if k == 1:
                    # contrarian contribution: gathered x rows weighted by
                    # the static selection matrix (coefc at (slot, 125+r))
                    nc.tensor.wait_ge(gather_sem, 16)
                    mm = nc.tensor.matmul(ctx_ps, selw_sb[0:16, :],
                                          gath_c[0:16, 0, :],
                                          start=False, stop=True)
                    add_dep_helper(mm.ins, st["gather"].ins,
                                   reason="sel matmul waits for gather")

                # gelu straight off PSUM (Act) into the bf16 out buffer;
                # the final *scale is applied on the host
                if not unit_affine:
                    z_t = blp.tile([PB, D], f32, tag="z")
                    nc.vector.tensor_mul(z_t, ctx_ps, gain_sb)
                    nc.vector.tensor_add(z_t, z_t, bias_sb)
                    gelu_src = z_t
                else:
                    gelu_src = ctx_ps
                if False:
                    pass
                else:
                    half = pos % 2
                    if half == 0:
                        pair_t = blp.tile([PB, 2, D], bf16, tag="pair",
                                          name="pair_t")
                        state["pair"] = pair_t
                    d_t = state["pair"]
                    nc.scalar.activation(d_t[:, half, :], gelu_src, Act.Gelu)
                    if half == 1:
                        lo = (pos - 1) * PB
                        nc.sync.dma_start(
                            out=out_ext[lo:lo + 2 * PB, :].rearrange(
                                "(c p) d -> p c d", p=PB),
                            in_=d_t)
                del state[k]

            # emission: all sims first (the in-order PE queue would
            # head-of-line block on any agg whose xbar transpose is still
            # in flight), selects interleaved, then all aggs
            for i in range(len(TORD)):
                stage_sim(TORD[i])
                if i >= 1:
                    stage_select(TORD[i - 1])
            stage_select(TORD[-1])
            for j in range(len(AGG_ORD)):
                stage_agg(AGG_ORD[j], j)

    nc.compile()
    return nc


def _get_program(unit_affine=True):
    key = bool(unit_affine)
    if key not in _PROGRAMS:
        _PROGRAMS[key] = _build_program(unit_affine=key)
    return _PROGRAMS[key]


def _make_in_maps(inputs):
    """Host-side prep: returns (in_maps for cores 0-7, unit_affine flag)."""
    x = np.asarray(inputs["x"], dtype=np.float32)
    gain = np.asarray(inputs["gain"], dtype=np.float32).reshape(D)
    bias = np.asarray(inputs["bias"], dtype=np.float32).reshape(D)
    log_mix = float(np.asarray(inputs["log_mix"]))
    log_alpha = float(np.asarray(inputs["log_alpha"]))
    log_scale = float(np.asarray(inputs["log_scale"]))

    mix = np.float32(1.0 / (1.0 + np.exp(-np.float64(log_mix))))
    alpha = np.float32(1.0 / (1.0 + np.exp(-np.float64(log_alpha))))
    scale = np.float32(np.logaddexp(0.0, np.float64(log_scale)) + 0.01)
    unit_affine = bool(np.all(gain == 1.0) and np.all(bias == 0.0))

    alpha1m = alpha * (np.float32(1.0) - mix)   # folded into shipped x
    consts = np.zeros((PB, 20), np.float32)
    consts[:, 0] = mix / alpha1m                # mixeye coefficient
    consts[:, 1] = np.float32(1.0) - mix
    consts[:, 2] = alpha * (np.float32(1.0) - mix)
    consts[:, 3] = (np.float32(1.0) - alpha) * (np.float32(1.0) - mix)
    consts[:, 4] = scale
    consts[:, 8:17] = np.arange(9, dtype=np.float32)[None, :]
    consts[:, 17] = (np.arange(PB) % 16).astype(np.float32)
    eye_bf = np.eye(PB, dtype=np.float32).astype(np.float16)
    patterns = _build_patterns()

    swap_perm = np.arange(NBLK).reshape(-1, 2)[:, ::-1].reshape(-1)

    # host-side normalize (f32) once per batch
    xn = x / np.maximum(np.linalg.norm(x, axis=-1, keepdims=True), 1e-12)

    in_maps = []
    for c in range(8):
        b, p = c // 2, c % 2
        xb = x[b]
        xnb = xn[b]
        if p:
            xb = xb.reshape(NBLK, PB, D)[swap_perm].reshape(T, D)
            xnb = xnb.reshape(NBLK, PB, D)[swap_perm].reshape(T, D)
        # xnT layout [dh, c, h, t]
        xnt = np.ascontiguousarray(
            xnb.reshape(NBLK, PB, 2, PB).transpose(3, 0, 2, 1)
            .reshape(PB, NBLK * 2 * PB).astype(np.float16))
        # contrarian selection weights: slot r*3+j (partition) -> column
        # 125+r gets (1-alpha)(1-mix)/(r+1) iff pick position j >= 2-r;
        # only even-parity cores own global rows 2045..2047
        selw = np.zeros((PB, PB), np.float32)
        if p == 0:
            for r in range(3):
                for j in range(3):
                    if j >= 2 - r:
                        selw[r * 3 + j, 125 + r] = (
                            (np.float32(1.0) - alpha)
                            * (np.float32(1.0) - mix)
                            / (np.float32(r + 1) * alpha1m))
        pateye = np.concatenate(
            [patterns[p].astype(np.float16).transpose(1, 0, 2).reshape(PB, -1),
             eye_bf, selw.astype(np.float16)], axis=1)
        im = {
            "x": np.ascontiguousarray((xb * alpha1m).astype(np.float16)),
            "xnt": xnt,
            "pateye": np.ascontiguousarray(pateye),
            "consts": consts,
        }
        if not unit_affine:
            im["gain_bc"] = np.ascontiguousarray(
                np.broadcast_to(gain[None, :], (PB, D)).astype(np.float32))
            im["bias_bc"] = np.ascontiguousarray(
                np.broadcast_to(bias[None, :], (PB, D)).astype(np.float32))
        in_maps.append(im)
    return in_maps, unit_affine


def kernel(**inputs):
    log_scale = float(np.asarray(inputs["log_scale"]))
    scale = np.float32(np.logaddexp(0.0, np.float64(log_scale)) + 0.01)
    in_maps, unit_affine = _make_in_maps(inputs)
    from concourse.bass_utils import run_bass_kernel_spmd
    nc = _get_program(unit_affine)
    res = run_bass_kernel_spmd(nc, in_maps, list(range(8))).results

    out = np.empty((B, T, D), np.float32)
    for c in range(8):
        b, p = c // 2, c % 2
        o = np.asarray(res[c]["out"]).astype(np.float32) * scale
        for i in range(NTILE):
            k = AGG_ORD[i]
            g_act = OWN[k] ^ p
            out[b, g_act * PB:(g_act + 1) * PB, :] = o[i * PB:(i + 1) * PB, :]
    return out
